# revision 6
# baseline (speedup 1.0000x reference)
"""FAGCN (FAConv stack) Trainium2 kernel, 8-core SPMD.

Sharding: 1D destination-node partition (6250 nodes/core), edges partitioned
by destination. Per layer each core gathers source rows [dinv*h | al] from a
replicated DRAM table (AllGather output) with dma_gather, computes
alpha = tanh(al_src + ar_dst) * dinv_src * dinv_dst on ACT/DVE, reduces
per-destination slot grids, and AllGathers the next layer's table.
"""
import sys

sys.path.insert(0, "/opt/trn_rl_repo")

import numpy as np
from concourse import bass, bacc, mybir, tile
from concourse.bass_utils import run_bass_kernel_spmd
from concourse.masks import make_identity

# problem constants
N = 50000
E = 800000
F_IN = 128
H = 64
L = 4
G = 64
EPS = 0.1
NC = 8
NPC = N // NC          # 6250 nodes per core
NBLK = (NPC + 127) // 128   # 49 blocks (last partial: 106 rows)
NPAD = NBLK * 128      # 6272
HALFN = N // 2         # table half split for int16 indices
DW = 128               # table row width in floats (512B): [h~ (64) | al | pad]
F32 = mybir.dt.float32
I16 = mybir.dt.int16


def _preprocess(edge_index):
    """Graph partitioning: renumber nodes per core by in-degree, build padded
    per-block slot grids (lo/hi table halves), wrapped int16 index arrays and
    mask*dinv grids. Pure index/structure work (independent of x)."""
    src = np.concatenate([np.asarray(edge_index[0], np.int64), np.arange(N)])
    dst = np.concatenate([np.asarray(edge_index[1], np.int64), np.arange(N)])
    deg = np.bincount(dst, minlength=N).astype(np.float32)
    dinv = (1.0 / np.sqrt(np.maximum(deg, 1.0))).astype(np.float32)

    newid = np.empty(N, np.int64)
    origid = np.empty(N, np.int64)
    for c in range(NC):
        b0 = c * NPC
        order = np.argsort(-deg[b0:b0 + NPC], kind="stable")
        newid[b0 + order] = b0 + np.arange(NPC)
        origid[b0 + np.arange(NPC)] = b0 + order
    srcn = newid[src]
    dstn = newid[dst]
    dinv_new = dinv[origid]

    cores = []
    # per-(core, block, half) counts -> uniform S across cores
    Sgrid = np.zeros((NC, NBLK, 2), np.int64)
    percore = []
    for c in range(NC):
        b0 = c * NPC
        m = (dstn >= b0) & (dstn < b0 + NPC)
        s_c = srcn[m]
        ld = dstn[m] - b0
        half = (s_c >= HALFN).astype(np.int64)
        key = ld * 2 + half
        cnts = np.bincount(key, minlength=NPC * 2)
        ordk = np.argsort(key, kind="stable")
        starts = np.concatenate([[0], np.cumsum(cnts)])
        ranks = np.empty(len(key), np.int64)
        ranks[ordk] = np.arange(len(key)) - np.repeat(starts[:-1], cnts)
        cnts_pad = np.zeros(NPAD * 2, np.int64)
        cnts_pad[:NPC * 2] = cnts
        cb = cnts_pad.reshape(NBLK, 128, 2)
        Sgrid[c] = cb.max(axis=1)
        percore.append((s_c, ld, half, ranks))

    S = Sgrid.max(axis=0)                      # [NBLK, 2] uniform slot counts
    S = np.maximum(S, 1)
    Ssum = S.sum(axis=1)                       # slots per block
    off = np.concatenate([[0], np.cumsum(Ssum)])  # column offsets per block
    ST = int(off[-1])                          # total slot columns per core

    for c in range(NC):
        s_c, ld, half, ranks = percore[c]
        b = ld // 128
        p = ld % 128
        col = off[b] + half * S[b, 0] + ranks
        gridI = np.zeros((128, ST), np.int64)
        gridM = np.zeros((128, ST), np.float32)
        gridI[p, col] = s_c - half * HALFN
        gridM[p, col] = dinv_new[c * NPC + ld]
        # wrapped int16 indices, per (block, half) piece, slot-major
        pieces = []
        for bb in range(NBLK):
            for hh in range(2):
                c0 = off[bb] + hh * S[bb, 0] if hh else off[bb]
                sw = S[bb, hh]
                flat = gridI[:, c0:c0 + sw].T.ravel()       # j = s*128+p
                wb = flat.reshape(8 * sw, 16).T.astype(np.int16)  # [16, 8S]
                pieces.append(np.tile(wb, (8, 1)))          # replicate to 128
        idxw = np.concatenate(pieces, axis=1)               # [128, 8*ST]
        cores.append(dict(idxw=idxw, maskdinv=gridM))

    # per-core dst-side dinv [128, NBLK] and orig ids
    ddst = np.zeros((NC, 128, NBLK), np.float32)
    for c in range(NC):
        dv = np.zeros(NPAD, np.float32)
        dv[:NPC] = dinv_new[c * NPC:(c + 1) * NPC]
        ddst[c] = dv.reshape(NBLK, 128).T
    return dict(S=S, off=off, ST=ST, cores=cores, ddst=ddst,
                origid=origid, dinv_new=dinv_new)


def _build_program(S, ST):
    """Bass program (identical across cores; per-core data differs)."""
    nc = bacc.Bacc(num_devices=NC)
    xT_in = nc.dram_tensor("xT", [128, NPAD], F32, kind="ExternalInput")
    w_in = nc.dram_tensor("w_in", [F_IN, H], F32, kind="ExternalInput")
    bin_in = nc.dram_tensor("bin_rep", [128, H], F32, kind="ExternalInput")
    w_out = nc.dram_tensor("w_out", [H, H], F32, kind="ExternalInput")
    bout_in = nc.dram_tensor("bout_rep", [128, H], F32, kind="ExternalInput")
    attl_in = nc.dram_tensor("attl_rep", [L * 128, H], F32, kind="ExternalInput")
    attr_in = nc.dram_tensor("attr_rep", [L * 128, H], F32, kind="ExternalInput")
    idx_in = nc.dram_tensor("idxw", [128, 8 * ST], I16, kind="ExternalInput")
    md_in = nc.dram_tensor("maskdinv", [128, ST], F32, kind="ExternalInput")
    ddst_in = nc.dram_tensor("ddst", [128, NBLK], F32, kind="ExternalInput")
    bh_in = nc.dram_tensor("bh", [128, NBLK * G], F32, kind="ExternalInput")
    hout = nc.dram_tensor("hout", [NPC, H], F32, kind="ExternalOutput")
    gpart = nc.dram_tensor("gpart", [G, H], F32, kind="ExternalOutput")

    Smax = int(S.sum(axis=1).max())
    rg = [list(range(NC))]

    with tile.TileContext(nc) as tc:
        with (
            tc.tile_pool(name="dram", bufs=1, space="DRAM") as dram,
            tc.tile_pool(name="persist", bufs=1) as pers,
            tc.tile_pool(name="work", bufs=3) as work,
            tc.tile_pool(name="gpool", bufs=3) as gpool,
            tc.tile_pool(name="psum", bufs=2, space="PSUM") as psum,
            tc.tile_pool(name="gppsum", bufs=1, space="PSUM") as gppsum,
        ):
            agin = dram.tile([NPC, DW], F32)
            Ts = [dram.tile([N, DW], F32, addr_space="Shared", name=f"T{l}", tag=f"T{l}")
                  for l in range(L)]

            # persistent SBUF state
            idxw = pers.tile([128, 8 * ST], I16)
            nc.sync.dma_start(idxw[:], idx_in[:])
            md = pers.tile([128, ST], F32)
            nc.sync.dma_start(md[:], md_in[:])
            ddst = pers.tile([128, NBLK], F32)
            nc.sync.dma_start(ddst[:], ddst_in[:])
            wi = pers.tile([F_IN, H], F32)
            nc.sync.dma_start(wi[:], w_in[:])
            wo = pers.tile([H, H], F32)
            nc.sync.dma_start(wo[:], w_out[:])
            bi = pers.tile([128, H], F32)
            nc.sync.dma_start(bi[:], bin_in[:])
            bo = pers.tile([128, H], F32)
            nc.sync.dma_start(bo[:], bout_in[:])
            attl = pers.tile([128, L * H], F32)
            attr = pers.tile([128, L * H], F32)
            for l in range(L):
                nc.sync.dma_start(attl[:, l * H:(l + 1) * H], attl_in[l * 128:(l + 1) * 128, :])
                nc.sync.dma_start(attr[:, l * H:(l + 1) * H], attr_in[l * 128:(l + 1) * 128, :])
            bh = pers.tile([128, NBLK * G], F32)
            nc.sync.dma_start(bh[:], bh_in[:])
            ident = pers.tile([128, 128], F32)
            make_identity(nc, ident[:])
            xT = pers.tile([128, NPAD], F32)
            nc.sync.dma_start(xT[:], xT_in[:])

            h_cur = pers.tile([128, NBLK * H], F32)
            eh0 = pers.tile([128, NBLK * H], F32)
            arcol = pers.tile([128, NBLK], F32)

            # zero the pad columns of agin once
            zt = pers.tile([128, DW - H - 1], F32)
            nc.gpsimd.memset(zt[:], 0.0)
            for b in range(NBLK):
                r0, r1 = b * 128, min((b + 1) * 128, NPC)
                nc.sync.dma_start(agin[r0:r1, H + 1:DW], zt[0:r1 - r0, :])

            def blk(t, b):
                return t[:, b * H:(b + 1) * H]

            def prep_block(b, l, hsrc):
                """table rows [dinv*h | al_l] for block b + arcol[:, b]."""
                tmp = work.tile([128, H], F32, tag="preptmp")
                nc.vector.tensor_tensor(tmp[:], hsrc, attr[:, l * H:(l + 1) * H],
                                        op=mybir.AluOpType.mult)
                nc.vector.tensor_reduce(out=arcol[:, b:b + 1], in_=tmp[:],
                                        axis=mybir.AxisListType.X,
                                        op=mybir.AluOpType.add)
                row = work.tile([128, H + 1], F32, tag="rowtile")
                nc.vector.tensor_tensor(tmp[:], hsrc, attl[:, l * H:(l + 1) * H],
                                        op=mybir.AluOpType.mult)
                nc.vector.tensor_reduce(out=row[:, H:H + 1], in_=tmp[:],
                                        axis=mybir.AxisListType.X,
                                        op=mybir.AluOpType.add)
                nc.vector.tensor_tensor(
                    row[:, 0:H], hsrc,
                    ddst[:, b:b + 1].broadcast_to([128, H]),
                    op=mybir.AluOpType.mult)
                r0, r1 = b * 128, min((b + 1) * 128, NPC)
                nc.sync.dma_start(agin[r0:r1, 0:H + 1], row[0:r1 - r0, :])

            # ---- input projection + layer-0 prep ----
            for b in range(NBLK):
                ps = psum.tile([128, H], F32, tag="proj")
                nc.tensor.matmul(ps[:], lhsT=xT[:, b * 128:(b + 1) * 128],
                                 rhs=wi[:], start=True, stop=True)
                tmp = work.tile([128, H], F32, tag="projtmp")
                nc.vector.tensor_tensor(tmp[:], ps[:], bi[:], op=mybir.AluOpType.add)
                nc.scalar.activation(blk(h_cur, b), tmp[:],
                                     mybir.ActivationFunctionType.Relu)
                prep_block(b, 0, blk(h_cur, b))
            nc.scalar.mul(eh0[:], h_cur[:], EPS)

            # ---- FAConv layers ----
            for l in range(L):
                T = Ts[l]
                nc.gpsimd.collective_compute(
                    "AllGather", mybir.AluOpType.bypass, replica_groups=rg,
                    ins=[agin[:]], outs=[T[:]])
                for b in range(NBLK):
                    slo, shi = int(S[b, 0]), int(S[b, 1])
                    ss = slo + shi
                    c0 = 0 if b == 0 else int(S[:b].sum())
                    gt = gpool.tile([128, Smax * DW], F32, tag="G")
                    gv = gt[:].rearrange("p (s d) -> p s d", d=DW)
                    SCH = 4  # slots (512 idxs) per dma_gather: SWDGE ring cap
                    for soff in range(0, slo, SCH):
                        sn = min(SCH, slo - soff)
                        nc.gpsimd.dma_gather(
                            out_ap=gv[:, soff:soff + sn, :], in_ap=T[0:HALFN, :],
                            idxs_ap=idxw[:, 8 * (c0 + soff):8 * (c0 + soff + sn)],
                            num_idxs=128 * sn, num_idxs_reg=128 * sn, elem_size=DW)
                    for soff in range(0, shi, SCH):
                        sn = min(SCH, shi - soff)
                        nc.gpsimd.dma_gather(
                            out_ap=gv[:, slo + soff:slo + soff + sn, :],
                            in_ap=T[HALFN:N, :],
                            idxs_ap=idxw[:, 8 * (c0 + slo + soff):8 * (c0 + slo + soff + sn)],
                            num_idxs=128 * sn, num_idxs_reg=128 * sn, elem_size=DW)
                    tt = work.tile([128, Smax], F32, tag="tt")
                    nc.scalar.activation(tt[:, 0:ss], gv[:, 0:ss, H],
                                         mybir.ActivationFunctionType.Tanh,
                                         bias=arcol[:, b:b + 1])
                    nc.vector.tensor_tensor(tt[:, 0:ss], tt[:, 0:ss],
                                            md[:, c0:c0 + ss],
                                            op=mybir.AluOpType.mult)
                    msg = work.tile([128, Smax * H], F32, tag="msg")
                    mv = msg[:].rearrange("p (s d) -> p s d", d=H)
                    nc.vector.tensor_tensor(
                        mv[:, 0:ss, :], gv[:, 0:ss, 0:H],
                        tt[:, 0:ss].unsqueeze(2).broadcast_to([128, ss, H]),
                        op=mybir.AluOpType.mult)
                    s = ss
                    while s > 1:
                        hh = s // 2
                        nc.vector.tensor_tensor(
                            mv[:, 0:hh, :], mv[:, 0:hh, :], mv[:, s - hh:s, :],
                            op=mybir.AluOpType.add)
                        s = s - hh
                    nc.vector.tensor_tensor(blk(h_cur, b), mv[:, 0, :],
                                            blk(eh0, b), op=mybir.AluOpType.add)
                    if l + 1 < L:
                        prep_block(b, l + 1, blk(h_cur, b))

            # ---- output projection + pooling ----
            gp = gppsum.tile([G, H], F32)
            for b in range(NBLK):
                tp = psum.tile([128, 128], F32, tag="trans")
                nc.tensor.transpose(out=tp[0:H, :], in_=blk(h_cur, b), identity=ident[:])
                hT = work.tile([H, 128], F32, tag="hT")
                nc.vector.tensor_copy(hT[:], tp[0:H, :])
                ps = psum.tile([128, H], F32, tag="proj")
                nc.tensor.matmul(ps[:], lhsT=hT[:], rhs=wo[:], start=True, stop=True)
                ho = work.tile([128, H], F32, tag="ho")
                nc.vector.tensor_tensor(ho[:], ps[:], bo[:], op=mybir.AluOpType.add)
                r0, r1 = b * 128, min((b + 1) * 128, NPC)
                nc.sync.dma_start(hout[r0:r1, :], ho[0:r1 - r0, :])
                nc.tensor.matmul(gp[:], lhsT=bh[:, b * G:(b + 1) * G], rhs=ho[:],
                                 start=(b == 0), stop=(b == NBLK - 1))
            gps = work.tile([G, H], F32, tag="gps")
            nc.vector.tensor_copy(gps[:], gp[:])
            nc.sync.dma_start(gpart[:], gps[:])

    nc.finalize()
    return nc


def prepare(x, edge_index, batch, W_in, b_in, att_l, att_r, W_out, b_out):
    x = np.asarray(x, np.float32)
    batch = np.asarray(batch, np.int64)
    W_in = np.asarray(W_in, np.float32)
    b_in = np.asarray(b_in, np.float32)
    att_l = np.asarray(att_l, np.float32)
    att_r = np.asarray(att_r, np.float32)
    W_out = np.asarray(W_out, np.float32)
    b_out = np.asarray(b_out, np.float32)

    meta = _preprocess(edge_index)
    S, ST, origid = meta["S"], meta["ST"], meta["origid"]

    nc = _build_program(S, ST)

    bin_rep = np.tile(b_in[None, :], (128, 1)).astype(np.float32)
    bout_rep = np.tile(b_out[None, :], (128, 1)).astype(np.float32)
    attl_rep = np.repeat(att_l, 128, axis=0).astype(np.float32)  # [L*128, H]
    attr_rep = np.repeat(att_r, 128, axis=0).astype(np.float32)

    in_maps = []
    for c in range(NC):
        oid = origid[c * NPC:(c + 1) * NPC]
        xT = np.zeros((128, NPAD), np.float32)
        xs = x[oid]                               # [NPC, F_IN]
        xT[:, :NPC] = xs.T
        bh = np.zeros((128, NBLK * G), np.float32)
        bg = batch[oid]                           # graph id per new row
        rows = np.arange(NPC)
        bh[rows % 128, (rows // 128) * G + bg] = 1.0
        in_maps.append(dict(
            xT=xT, w_in=W_in, bin_rep=bin_rep, w_out=W_out, bout_rep=bout_rep,
            attl_rep=attl_rep, attr_rep=attr_rep,
            idxw=np.ascontiguousarray(meta["cores"][c]["idxw"]),
            maskdinv=np.ascontiguousarray(meta["cores"][c]["maskdinv"]),
            ddst=np.ascontiguousarray(meta["ddst"][c]),
            bh=bh,
        ))

    return nc, in_maps, meta


def postprocess(results, meta):
    origid = meta["origid"]
    h_full = np.empty((N, H), np.float32)
    gemb = np.zeros((G, H), np.float32)
    for c in range(NC):
        oid = origid[c * NPC:(c + 1) * NPC]
        h_full[oid] = results[c]["hout"]
        gemb += results[c]["gpart"]
    return (gemb, h_full)


def kernel(x, edge_index, batch, W_in, b_in, att_l, att_r, W_out, b_out):
    nc, in_maps, meta = prepare(x, edge_index, batch, W_in, b_in,
                                att_l, att_r, W_out, b_out)
    res = run_bass_kernel_spmd(nc, in_maps, core_ids=list(range(NC)))
    if res.exec_time_ns is not None:
        print(f"HW exec time: {res.exec_time_ns} ns")
    return postprocess(res.results, meta)
